# revision 38
# baseline (speedup 1.0000x reference)
"""Llama attention layer (B=2, S=2048, D=2048, H=16, HD=128, RoPE, causal)
on 8 Trainium2 NeuronCores.

Sharding: core c -> (batch b = c//4, head group g = c%4 of 4 heads).
Each core computes q/k/v projections for its 512 columns of wq/wk/wv,
RoPE, causal attention for its 4 heads, and the out-projection against
its 512 rows of wo (a partial sum over head groups). The host sums the
4 partials per batch and stacks the 2 batches.

All device matmuls run in bf16 with fp32 PSUM accumulation. Softmax is
computed without max-subtraction (scores here are bounded ~|9|), with
the denominator obtained from an M=1 ones-matmul over exp(scores^T).
"""

import os
import sys

import numpy as np
import ml_dtypes

if "/opt/trn_rl_repo" not in sys.path:
    sys.path.insert(0, "/opt/trn_rl_repo")

import concourse.bass as bass  # noqa: E402
import concourse.mybir as mybir  # noqa: E402
import concourse.bacc as bacc  # noqa: E402
import concourse.tile as tile  # noqa: E402

BF16 = ml_dtypes.bfloat16

B, S, D, H = 2, 2048, 2048, 16
HD = D // H            # 128, head dim
G = 4                  # head groups (cores per batch)
NH = H // G            # 4 heads per core
DG = NH * HD           # 512, per-core head width
P = 128
KO = D // P            # 16 k-subtiles over D
NKT = S // P           # 16 key chunks of 128
NQT = S // 512         # 4 q tiles of 512
QT = 512
ROPE_THETA = 10000.0
SCALE = 1.0 / float(np.sqrt(HD))

N_CORES = 8

_BUILT = None  # (nc,) cache


def build_module():
    fp32 = mybir.dt.float32
    bf16 = mybir.dt.bfloat16

    nc = bacc.Bacc("TRN2", target_bir_lowering=False, debug=False,
                   num_devices=N_CORES, num_swdge_queues=4)

    xT = nc.dram_tensor("xT", [P, KO, S], bf16, kind="ExternalInput")
    wq = nc.dram_tensor("wq", [P, KO, DG], bf16, kind="ExternalInput")
    wk = nc.dram_tensor("wk", [P, KO, DG], bf16, kind="ExternalInput")
    wv = nc.dram_tensor("wv", [P, KO, DG], bf16, kind="ExternalInput")
    wo = nc.dram_tensor("wo", [P, NH, D], bf16, kind="ExternalInput")
    cosT = nc.dram_tensor("cosT", [P, S], bf16, kind="ExternalInput")
    sinT = nc.dram_tensor("sinT", [P, S], bf16, kind="ExternalInput")
    maskT = nc.dram_tensor("maskT", [P, NH, QT], bf16, kind="ExternalInput")
    out = nc.dram_tensor("out", [P, NKT, D], fp32, kind="ExternalOutput")

    Exp = mybir.ActivationFunctionType.Exp

    with tile.TileContext(nc) as tc:
        with tc.tile_pool(name="const", bufs=1) as const, \
             tc.tile_pool(name="big", bufs=1) as big:
            ones = const.tile([P, P], bf16)
            nc.vector.memset(ones, 1.0)
            # dummy exp so the ACT Exp table loads during the DMA prefix,
            # not at the first real exp in the attention phase
            warm = const.tile([1, 1], fp32)
            nc.scalar.activation(warm, ones[0:1, 0:1],
                                 mybir.ActivationFunctionType.Exp)

            qT_sb = big.tile([P, NH, S], bf16)   # per head: [HD, S]
            kT_sb = big.tile([P, NH, S], bf16)
            v_sb = big.tile([P, NKT, DG], bf16)  # [key%128, keychunk, dg]

            # ---------------- phase 1: projections + RoPE ----------------
            with tc.tile_pool(name="w_pool", bufs=1) as w_pool, \
                 tc.tile_pool(name="rope", bufs=3) as rope, \
                 tc.tile_pool(name="ps1", bufs=1, space="PSUM") as ps1:
                # DMA order matters: wv first (V-loop gate), then xT, then
                # the rest (not needed until the V-loop is done). Spread the
                # issues over several engine queues so they don't serialize
                # behind one sequencer.
                wv_sb = w_pool.tile_from(wv.ap())
                xT_sb = w_pool.tile([P, KO, S], bf16)
                for k in range(KO):
                    nc.sync.dma_start(xT_sb[:, k, :], xT.ap()[:, k, :])
                wk_sb = w_pool.tile_from(wk.ap())
                wq_sb = w_pool.tile_from(wq.ap())
                cos_sb = w_pool.tile_from(cosT.ap())
                sin_sb = w_pool.tile_from(sinT.ap())

                # V: [keys, dg] natural layout, keychunk tiles of 128.
                # bufs=4 so several m-groups advance in lockstep while the
                # xT chunks stream in.
                for m in range(NKT):
                    ps = ps1.tile([P, DG], fp32, tag="psv", bufs=4)
                    for k in range(KO):
                        nc.tensor.matmul(ps, xT_sb[:, k, m * P:(m + 1) * P],
                                         wv_sb[:, k, :],
                                         start=(k == 0), stop=(k == KO - 1))
                    nc.vector.tensor_copy(v_sb[:, m, :], ps)

                # K then Q: [HD, S] transposed layout + RoPE.
                # Heads processed in pairs so the two psum tags can be
                # double-buffered (2 tags x 2 bufs) -- RoPE of one pair
                # overlaps the matmuls of the next.
                for which, w_sb, dstT in (("k", wk_sb, kT_sb), ("q", wq_sb, qT_sb)):
                    for nt2 in range(2 * NQT):
                        nt, hp = divmod(nt2, 2)
                        sl = slice(nt * QT, (nt + 1) * QT)
                        heads = (2 * hp, 2 * hp + 1)
                        # share the "psv" tag (banks 0-3) so all of phase 1
                        # stays within 4 psum banks, leaving 4-7 free for
                        # the attention pools to start without bank conflicts
                        pss = {}
                        for h in heads:
                            pss[h] = ps1.tile([P, QT], fp32, tag="psv",
                                              name=f"psp{h}", bufs=4)
                        for k in range(KO):
                            for h in heads:
                                nc.tensor.matmul(
                                    pss[h], w_sb[:, k, h * HD:(h + 1) * HD],
                                    xT_sb[:, k, sl],
                                    start=(k == 0), stop=(k == KO - 1))
                        for h in heads:
                            ps = pss[h]
                            dst = dstT[:, h, sl]
                            # rope: dst = ps * cos + swap(ps) * sin_signed.
                            # The swapped reads must come from PSUM (the SB-SB
                            # same-base-partition rule forbids them on SBUF);
                            # the straight read goes via a parallel ACT copy so
                            # the psum bank drains fast.
                            tmp = rope.tile([P, QT], bf16, tag="tmp")
                            nc.vector.tensor_mul(tmp[0:64], ps[64:128],
                                                 sin_sb[0:64, sl])
                            nc.vector.tensor_mul(tmp[64:128], ps[0:64],
                                                 sin_sb[64:128, sl])
                            qb = rope.tile([P, QT], bf16, tag="qb")
                            nc.scalar.copy(qb, ps)
                            nc.vector.tensor_mul(dst, qb, cos_sb[:, sl])
                            nc.vector.tensor_add(dst, dst, tmp)

            # ---------------- phases 2+3 ----------------
            with tc.tile_pool(name="big2", bufs=1) as big2:
                aoT_sb = big2.tile([P, NH, S], bf16)  # attention out^T
                mask_sb = const.tile_from(maskT.ap())
                wo_sb = big2.tile([P, NH, D], bf16)
                nc.sync.dma_start(wo_sb, wo.ap())

                # phases 2+3 interleaved: attention for q-tile qt, then the
                # out-projection rows it unblocks (their matmuls have no ACT
                # dependency and fill the exp-latency bubbles)
                with tc.tile_pool(name="ax_pool", bufs=16) as ax_pool, \
                     tc.tile_pool(name="ep", bufs=3) as ep, \
                     tc.tile_pool(name="stage", bufs=4) as stage, \
                     tc.tile_pool(name="ps2s", bufs=3, space="PSUM") as ps2s, \
                     tc.tile_pool(name="ps2o", bufs=2, space="PSUM") as ps2o, \
                     tc.tile_pool(name="ps2", bufs=1, space="PSUM") as ps2, \
                     tc.tile_pool(name="ps3", bufs=2, space="PSUM") as ps3:
                    for qt in range(NQT):
                        qsl = slice(qt * QT, (qt + 1) * QT)
                        n_kt = 4 * (qt + 1)  # causal: key chunks 0..n_kt-1
                        for h in range(NH):
                            ps_o = ps2o.tile([P, QT], fp32, tag="ps_o")
                            # all-ones [128,128] lhsT -> every psum row holds
                            # sumexp: no partition-broadcast needed later
                            ps_sum = ps2.tile([P, QT], fp32, tag="ps_sum")
                            for m in range(n_kt):
                                ps_s = ps2s.tile([P, QT], fp32, tag="ps_s")
                                nc.tensor.matmul(ps_s,
                                                 kT_sb[:, h, m * P:(m + 1) * P],
                                                 qT_sb[:, h, qsl],
                                                 start=True, stop=True)
                                ax = ax_pool.tile([P, QT], bf16, tag="ax")
                                nc.scalar.activation(ax, ps_s, Exp, scale=SCALE)
                                if m >= qt * 4:
                                    nc.vector.tensor_mul(
                                        ax, ax, mask_sb[:, m - qt * 4, :])
                                nc.tensor.matmul(
                                    ps_o, v_sb[:, m, h * HD:(h + 1) * HD],
                                    ax, start=(m == 0), stop=(m == n_kt - 1))
                                nc.tensor.matmul(ps_sum, ones, ax,
                                                 start=(m == 0),
                                                 stop=(m == n_kt - 1))
                            rec = ep.tile([P, QT], fp32, tag="rec")
                            nc.vector.reciprocal_approx_fast(rec, ps_sum)
                            nc.vector.tensor_mul(aoT_sb[:, h, qsl], ps_o, rec)

                        # out-projection for the q rows finished by this qt
                        for qo in range(4 * qt, 4 * (qt + 1)):
                            for n in range(D // QT):
                                nsl = slice(n * QT, (n + 1) * QT)
                                ps = ps3.tile([P, QT], fp32, tag="ps_out")
                                for h in range(NH):
                                    nc.tensor.matmul(
                                        ps, aoT_sb[:, h, qo * P:(qo + 1) * P],
                                        wo_sb[:, h, nsl],
                                        start=(h == 0), stop=(h == NH - 1))
                                ob = stage.tile([P, QT], fp32, tag="ob")
                                nc.vector.tensor_copy(ob, ps)
                                nc.sync.dma_start(out.ap()[:, qo, nsl], ob)

    nc.compile()
    return nc


def _rope_tables():
    inv_freq = 1.0 / (ROPE_THETA ** (np.arange(0, HD, 2, dtype=np.float64) / HD))
    pos = np.arange(S, dtype=np.float64)
    freqs = np.outer(pos, inv_freq)                    # [S, HD/2]
    emb = np.concatenate([freqs, freqs], axis=-1)      # [S, HD]
    cos = np.cos(emb).T.astype(BF16)                   # [HD, S]
    sin = np.sin(emb).T.astype(np.float32)
    sin[: HD // 2] *= -1.0                             # fold rotate_half sign
    return cos, sin.astype(BF16)


def _pack_kd(a):
    """[D, N] -> [P, D//P, N] with d = ko*P + p."""
    d, n = a.shape
    return np.ascontiguousarray(
        a.reshape(d // P, P, n).transpose(1, 0, 2)).astype(BF16)


def make_in_maps(x, wq, wk, wv, wo):
    cosT, sinT = _rope_tables()
    mask = np.zeros((P, NH, QT), dtype=BF16)
    for o in range(NH):
        i = np.arange(P)[:, None]
        j = np.arange(QT)[None, :]
        mask[:, o, :] = (o * P + i <= j).astype(BF16)

    in_maps = []
    for c in range(N_CORES):
        b, g = divmod(c, G)
        gsl = slice(g * DG, (g + 1) * DG)
        in_maps.append({
            "xT": _pack_kd(np.ascontiguousarray(x[b].T)),
            "wq": _pack_kd(wq[:, gsl]),
            "wk": _pack_kd(wk[:, gsl]),
            "wv": _pack_kd(wv[:, gsl]),
            "wo": _pack_kd(np.ascontiguousarray(wo[gsl, :])),
            "cosT": cosT,
            "sinT": sinT,
            "maskT": mask,
        })
    return in_maps


def assemble_output(results):
    """results: list of 8 dicts with 'out' [P, NKT, D] fp32."""
    full = np.empty((B, S, D), dtype=np.float32)
    for b in range(B):
        acc = None
        for g in range(G):
            r = results[b * G + g]["out"]
            part = r.transpose(1, 0, 2).reshape(S, D)
            acc = part.copy() if acc is None else acc + part
        full[b] = acc
    return full


def _get_module():
    global _BUILT
    if _BUILT is None:
        _BUILT = build_module()
    return _BUILT


def _install_trace_shim():
    """This image's antenv lacks axon_hooks; provide the NTFF profile hook
    via ctypes so trace=True (or BASS_TRACE=1) works instead of crashing,
    and skip the artifact bucket upload."""
    try:
        import antenv.axon_hooks  # noqa: F401
        return
    except ImportError:
        pass
    import types
    import ctypes
    import contextlib

    so_path = "/opt/axon/libaxon_pjrt.so"
    mod = types.ModuleType("antenv.axon_hooks")
    try:
        lib = ctypes.CDLL(so_path)
        lib.axon_start_nrt_profile.argtypes = [
            ctypes.POINTER(ctypes.c_int64), ctypes.c_size_t]
        lib.axon_start_nrt_profile.restype = ctypes.c_int64
        lib.axon_stop_nrt_profile.argtypes = [ctypes.c_char_p]
        lib.axon_stop_nrt_profile.restype = ctypes.c_int64

        @contextlib.contextmanager
        def _hook(output_dir, device_ids):
            import jax
            jax.devices()
            if device_ids:
                ids = (ctypes.c_int64 * len(device_ids))(*device_ids)
                rc = lib.axon_start_nrt_profile(ids, len(device_ids))
            else:
                rc = lib.axon_start_nrt_profile(None, 0)
            if rc != 0:
                raise RuntimeError(f"axon_start_nrt_profile rc={rc}")
            try:
                yield
            finally:
                lib.axon_stop_nrt_profile(str(output_dir).encode())

        mod.get_axon_ntff_profile_hook = lambda: _hook
    except OSError:
        mod.get_axon_ntff_profile_hook = lambda: None
    mod.set_axon_ntff_profile_hook = lambda h: None
    sys.modules["antenv.axon_hooks"] = mod

    from concourse import bass_utils
    bass_utils.upload_artifacts = lambda tmpdir: tmpdir


def run_on_hw(in_maps, trace=False, trace_cores=None):
    _install_trace_shim()
    from concourse import bass_utils
    nc = _get_module()
    return bass_utils.run_bass_kernel_spmd(
        nc, in_maps, core_ids=list(range(N_CORES)),
        trace=trace, trace_cores=trace_cores)


def kernel(x, wq, wk, wv, wo):
    x = np.asarray(x, dtype=np.float32)
    wq = np.asarray(wq, dtype=np.float32)
    wk = np.asarray(wk, dtype=np.float32)
    wv = np.asarray(wv, dtype=np.float32)
    wo = np.asarray(wo, dtype=np.float32)
    in_maps = make_in_maps(x, wq, wk, wv, wo)
    res = run_on_hw(in_maps, trace=False)
    return assemble_output(res.results)


# revision 41
# speedup vs baseline: 1.0709x; 1.0709x over previous
"""Llama attention layer (B=2, S=2048, D=2048, H=16, HD=128, RoPE, causal)
on 8 Trainium2 NeuronCores.

Sharding: core c -> (batch b = c//4, head group g = c%4 of 4 heads).
Each core computes q/k/v projections for its 512 columns of wq/wk/wv,
RoPE, causal attention for its 4 heads, and the out-projection against
its 512 rows of wo (a partial sum over head groups). The host sums the
4 partials per batch and stacks the 2 batches.

All device matmuls run in bf16 with fp32 PSUM accumulation. Softmax is
computed without max-subtraction (scores here are bounded ~|9|), with
the denominator obtained from an M=1 ones-matmul over exp(scores^T).
"""

import os
import sys

import numpy as np
import ml_dtypes

if "/opt/trn_rl_repo" not in sys.path:
    sys.path.insert(0, "/opt/trn_rl_repo")

import concourse.bass as bass  # noqa: E402
import concourse.mybir as mybir  # noqa: E402
import concourse.bacc as bacc  # noqa: E402
import concourse.tile as tile  # noqa: E402

BF16 = ml_dtypes.bfloat16

B, S, D, H = 2, 2048, 2048, 16
HD = D // H            # 128, head dim
G = 4                  # head groups (cores per batch)
NH = H // G            # 4 heads per core
DG = NH * HD           # 512, per-core head width
P = 128
KO = D // P            # 16 k-subtiles over D
NKT = S // P           # 16 key chunks of 128
NQT = S // 512         # 4 q tiles of 512
QT = 512
ROPE_THETA = 10000.0
SCALE = 1.0 / float(np.sqrt(HD))

N_CORES = 8

_BUILT = None  # (nc,) cache


def build_module():
    fp32 = mybir.dt.float32
    bf16 = mybir.dt.bfloat16

    nc = bacc.Bacc("TRN2", target_bir_lowering=False, debug=False,
                   num_devices=N_CORES, num_swdge_queues=4)

    xT = nc.dram_tensor("xT", [P, KO, S], bf16, kind="ExternalInput")
    wq = nc.dram_tensor("wq", [P, KO, DG], bf16, kind="ExternalInput")
    wk = nc.dram_tensor("wk", [P, KO, DG], bf16, kind="ExternalInput")
    wv = nc.dram_tensor("wv", [P, KO, DG], bf16, kind="ExternalInput")
    wo = nc.dram_tensor("wo", [P, NH, D], bf16, kind="ExternalInput")
    cosT = nc.dram_tensor("cosT", [P, S], bf16, kind="ExternalInput")
    sinT = nc.dram_tensor("sinT", [P, S], bf16, kind="ExternalInput")
    maskT = nc.dram_tensor("maskT", [P, NH, QT], bf16, kind="ExternalInput")
    out = nc.dram_tensor("out", [P, NKT, D], fp32, kind="ExternalOutput")

    Exp = mybir.ActivationFunctionType.Exp

    with tile.TileContext(nc) as tc:
        with tc.tile_pool(name="const", bufs=1) as const, \
             tc.tile_pool(name="big", bufs=1) as big:
            ones = const.tile([P, P], bf16)
            nc.vector.memset(ones, 1.0)
            # dummy exp so the ACT Exp table loads during the DMA prefix,
            # not at the first real exp in the attention phase
            warm = const.tile([1, 1], fp32)
            nc.scalar.activation(warm, ones[0:1, 0:1],
                                 mybir.ActivationFunctionType.Exp)

            qT_sb = big.tile([P, NH, S], bf16)   # per head: [HD, S]
            kT_sb = big.tile([P, NH, S], bf16)
            v_sb = big.tile([P, NKT, DG], bf16)  # [key%128, keychunk, dg]

            # ---------------- phase 1: projections + RoPE ----------------
            with tc.tile_pool(name="w_pool", bufs=1) as w_pool, \
                 tc.tile_pool(name="rope", bufs=3) as rope, \
                 tc.tile_pool(name="ps1", bufs=1, space="PSUM") as ps1:
                # DMA order matters: wv first (V-loop gate), then xT, then
                # the rest (not needed until the V-loop is done). Spread the
                # issues over several engine queues so they don't serialize
                # behind one sequencer.
                wv_sb = w_pool.tile_from(wv.ap())
                xT_sb = w_pool.tile([P, KO, S], bf16)
                for k in range(KO):
                    nc.sync.dma_start(xT_sb[:, k, :], xT.ap()[:, k, :])
                wk_sb = w_pool.tile_from(wk.ap())
                wq_sb = w_pool.tile_from(wq.ap())
                cos_sb = w_pool.tile_from(cosT.ap())
                sin_sb = w_pool.tile_from(sinT.ap())

                # V: [keys, dg] natural layout, keychunk tiles of 128.
                # bufs=4 so several m-groups advance in lockstep while the
                # xT chunks stream in.
                for m in range(NKT):
                    ps = ps1.tile([P, DG], fp32, tag="psv", bufs=4)
                    for k in range(KO):
                        nc.tensor.matmul(ps, xT_sb[:, k, m * P:(m + 1) * P],
                                         wv_sb[:, k, :],
                                         start=(k == 0), stop=(k == KO - 1))
                    nc.vector.tensor_copy(v_sb[:, m, :], ps)

                # K then Q: [HD, S] transposed layout + RoPE.
                # Heads processed in pairs so the two psum tags can be
                # double-buffered (2 tags x 2 bufs) -- RoPE of one pair
                # overlaps the matmuls of the next.
                for which, w_sb, dstT in (("k", wk_sb, kT_sb), ("q", wq_sb, qT_sb)):
                    for nt2 in range(2 * NQT):
                        nt, hp = divmod(nt2, 2)
                        sl = slice(nt * QT, (nt + 1) * QT)
                        heads = (2 * hp, 2 * hp + 1)
                        # share the "psv" tag (banks 0-3) so all of phase 1
                        # stays within 4 psum banks, leaving 4-7 free for
                        # the attention pools to start without bank conflicts
                        pss = {}
                        for h in heads:
                            pss[h] = ps1.tile([P, QT], fp32, tag="psv",
                                              name=f"psp{h}", bufs=4)
                        for k in range(KO):
                            for h in heads:
                                nc.tensor.matmul(
                                    pss[h], w_sb[:, k, h * HD:(h + 1) * HD],
                                    xT_sb[:, k, sl],
                                    start=(k == 0), stop=(k == KO - 1))
                        for h in heads:
                            ps = pss[h]
                            dst = dstT[:, h, sl]
                            # rope: dst = ps * cos + swap(ps) * sin_signed.
                            # The swapped reads must come from PSUM (the SB-SB
                            # same-base-partition rule forbids them on SBUF);
                            # the straight read goes via a parallel ACT copy so
                            # the psum bank drains fast.
                            tmp = rope.tile([P, QT], bf16, tag="tmp")
                            nc.vector.tensor_mul(tmp[0:64], ps[64:128],
                                                 sin_sb[0:64, sl])
                            nc.vector.tensor_mul(tmp[64:128], ps[0:64],
                                                 sin_sb[64:128, sl])
                            qb = rope.tile([P, QT], bf16, tag="qb")
                            nc.scalar.copy(qb, ps)
                            nc.vector.tensor_mul(dst, qb, cos_sb[:, sl])
                            nc.vector.tensor_add(dst, dst, tmp)

            # ---------------- phases 2+3 ----------------
            with tc.tile_pool(name="big2", bufs=1) as big2:
                aoT_sb = big2.tile([P, NH, S], bf16)  # attention out^T
                mask_sb = const.tile_from(maskT.ap())
                wo_sb = big2.tile([P, NH, D], bf16)
                nc.sync.dma_start(wo_sb, wo.ap())

                # phases 2+3 interleaved: attention for q-tile qt, then the
                # out-projection rows it unblocks (their matmuls have no ACT
                # dependency and fill the exp-latency bubbles)
                with tc.tile_pool(name="ax_pool", bufs=16) as ax_pool, \
                     tc.tile_pool(name="ep", bufs=3) as ep, \
                     tc.tile_pool(name="stage", bufs=4) as stage, \
                     tc.tile_pool(name="ps2s", bufs=3, space="PSUM") as ps2s, \
                     tc.tile_pool(name="ps2o", bufs=2, space="PSUM") as ps2o, \
                     tc.tile_pool(name="ps2", bufs=1, space="PSUM") as ps2, \
                     tc.tile_pool(name="ps3", bufs=2, space="PSUM") as ps3:
                    for qt in range(NQT):
                        qsl = slice(qt * QT, (qt + 1) * QT)
                        n_kt = 4 * (qt + 1)  # causal: key chunks 0..n_kt-1
                        for h in range(NH):
                            ps_o = ps2o.tile([P, QT], fp32, tag="ps_o")
                            # all-ones [128,128] lhsT -> every psum row holds
                            # sumexp: no partition-broadcast needed later
                            ps_sum = ps2.tile([P, QT], fp32, tag="ps_sum")
                            ax_prev = None
                            for m in range(n_kt):
                                ps_s = ps2s.tile([P, QT], fp32, tag="ps_s")
                                nc.tensor.matmul(ps_s,
                                                 kT_sb[:, h, m * P:(m + 1) * P],
                                                 qT_sb[:, h, qsl],
                                                 start=True, stop=True)
                                ax = ax_pool.tile([P, QT], bf16, tag="ax")
                                nc.scalar.activation(ax, ps_s, Exp, scale=SCALE)
                                if m >= qt * 4:
                                    nc.vector.tensor_mul(
                                        ax, ax, mask_sb[:, m - qt * 4, :])
                                nc.tensor.matmul(
                                    ps_o, v_sb[:, m, h * HD:(h + 1) * HD],
                                    ax, start=(m == 0), stop=(m == n_kt - 1))
                                # denominator: pre-add exp pairs on DVE (off
                                # the V-matmul chain) -> half the ones-matmuls
                                if m % 2 == 0:
                                    ax_prev = ax
                                else:
                                    axs = ax_pool.tile([P, QT], bf16, tag="axs")
                                    nc.vector.tensor_add(axs, ax_prev, ax)
                                    nc.tensor.matmul(ps_sum, ones, axs,
                                                     start=(m == 1),
                                                     stop=(m == n_kt - 1))
                            rec = ep.tile([P, QT], fp32, tag="rec")
                            nc.vector.reciprocal_approx_fast(rec, ps_sum)
                            nc.vector.tensor_mul(aoT_sb[:, h, qsl], ps_o, rec)

                        # out-projection for the q rows finished by this qt
                        for qo in range(4 * qt, 4 * (qt + 1)):
                            for n in range(D // QT):
                                nsl = slice(n * QT, (n + 1) * QT)
                                ps = ps3.tile([P, QT], fp32, tag="ps_out")
                                for h in range(NH):
                                    nc.tensor.matmul(
                                        ps, aoT_sb[:, h, qo * P:(qo + 1) * P],
                                        wo_sb[:, h, nsl],
                                        start=(h == 0), stop=(h == NH - 1))
                                ob = stage.tile([P, QT], fp32, tag="ob")
                                nc.vector.tensor_copy(ob, ps)
                                nc.sync.dma_start(out.ap()[:, qo, nsl], ob)

    nc.compile()
    return nc


def _rope_tables():
    inv_freq = 1.0 / (ROPE_THETA ** (np.arange(0, HD, 2, dtype=np.float64) / HD))
    pos = np.arange(S, dtype=np.float64)
    freqs = np.outer(pos, inv_freq)                    # [S, HD/2]
    emb = np.concatenate([freqs, freqs], axis=-1)      # [S, HD]
    cos = np.cos(emb).T.astype(BF16)                   # [HD, S]
    sin = np.sin(emb).T.astype(np.float32)
    sin[: HD // 2] *= -1.0                             # fold rotate_half sign
    return cos, sin.astype(BF16)


def _pack_kd(a):
    """[D, N] -> [P, D//P, N] with d = ko*P + p."""
    d, n = a.shape
    return np.ascontiguousarray(
        a.reshape(d // P, P, n).transpose(1, 0, 2)).astype(BF16)


def make_in_maps(x, wq, wk, wv, wo):
    cosT, sinT = _rope_tables()
    mask = np.zeros((P, NH, QT), dtype=BF16)
    for o in range(NH):
        i = np.arange(P)[:, None]
        j = np.arange(QT)[None, :]
        mask[:, o, :] = (o * P + i <= j).astype(BF16)

    in_maps = []
    for c in range(N_CORES):
        b, g = divmod(c, G)
        gsl = slice(g * DG, (g + 1) * DG)
        in_maps.append({
            "xT": _pack_kd(np.ascontiguousarray(x[b].T)),
            "wq": _pack_kd(wq[:, gsl]),
            "wk": _pack_kd(wk[:, gsl]),
            "wv": _pack_kd(wv[:, gsl]),
            "wo": _pack_kd(np.ascontiguousarray(wo[gsl, :])),
            "cosT": cosT,
            "sinT": sinT,
            "maskT": mask,
        })
    return in_maps


def assemble_output(results):
    """results: list of 8 dicts with 'out' [P, NKT, D] fp32."""
    full = np.empty((B, S, D), dtype=np.float32)
    for b in range(B):
        acc = None
        for g in range(G):
            r = results[b * G + g]["out"]
            part = r.transpose(1, 0, 2).reshape(S, D)
            acc = part.copy() if acc is None else acc + part
        full[b] = acc
    return full


def _get_module():
    global _BUILT
    if _BUILT is None:
        _BUILT = build_module()
    return _BUILT


def _install_trace_shim():
    """This image's antenv lacks axon_hooks; provide the NTFF profile hook
    via ctypes so trace=True (or BASS_TRACE=1) works instead of crashing,
    and skip the artifact bucket upload."""
    try:
        import antenv.axon_hooks  # noqa: F401
        return
    except ImportError:
        pass
    import types
    import ctypes
    import contextlib

    so_path = "/opt/axon/libaxon_pjrt.so"
    mod = types.ModuleType("antenv.axon_hooks")
    try:
        lib = ctypes.CDLL(so_path)
        lib.axon_start_nrt_profile.argtypes = [
            ctypes.POINTER(ctypes.c_int64), ctypes.c_size_t]
        lib.axon_start_nrt_profile.restype = ctypes.c_int64
        lib.axon_stop_nrt_profile.argtypes = [ctypes.c_char_p]
        lib.axon_stop_nrt_profile.restype = ctypes.c_int64

        @contextlib.contextmanager
        def _hook(output_dir, device_ids):
            import jax
            jax.devices()
            if device_ids:
                ids = (ctypes.c_int64 * len(device_ids))(*device_ids)
                rc = lib.axon_start_nrt_profile(ids, len(device_ids))
            else:
                rc = lib.axon_start_nrt_profile(None, 0)
            if rc != 0:
                raise RuntimeError(f"axon_start_nrt_profile rc={rc}")
            try:
                yield
            finally:
                lib.axon_stop_nrt_profile(str(output_dir).encode())

        mod.get_axon_ntff_profile_hook = lambda: _hook
    except OSError:
        mod.get_axon_ntff_profile_hook = lambda: None
    mod.set_axon_ntff_profile_hook = lambda h: None
    sys.modules["antenv.axon_hooks"] = mod

    from concourse import bass_utils
    bass_utils.upload_artifacts = lambda tmpdir: tmpdir


def run_on_hw(in_maps, trace=False, trace_cores=None):
    _install_trace_shim()
    from concourse import bass_utils
    nc = _get_module()
    return bass_utils.run_bass_kernel_spmd(
        nc, in_maps, core_ids=list(range(N_CORES)),
        trace=trace, trace_cores=trace_cores)


def kernel(x, wq, wk, wv, wo):
    x = np.asarray(x, dtype=np.float32)
    wq = np.asarray(wq, dtype=np.float32)
    wk = np.asarray(wk, dtype=np.float32)
    wv = np.asarray(wv, dtype=np.float32)
    wo = np.asarray(wo, dtype=np.float32)
    in_maps = make_in_maps(x, wq, wk, wv, wo)
    res = run_on_hw(in_maps, trace=False)
    return assemble_output(res.results)
